# revision 21
# baseline (speedup 1.0000x reference)
"""MC-CNN stereo cost-volume inference on 8 Trainium2 NeuronCores.

Sharding: image rows. Core k computes output feature rows [24k, 24k+24)
(input image rows [24k, 24k+34) with conv halo). Each core runs the full
5-layer conv tower on its slice, then the 4-layer MLP for all 32
disparities. Output rows are concatenated on the host.

Key structural tricks:
  - Feature-major layout [C, rows*W] with flat-sliding conv taps; the 2
    garbage columns per row per layer stay right of the valid region.
  - MLP layer 1 is hoisted out of the disparity loop:
      h1_d = relu(fw1L.T@lf + shift(fw1R.T@rf, d))
    L1 = fw1L.T@lf and R1 = fw1R.T@rf are computed once per row-block;
    the shift is a flat offset read of R1 (garbage cols absorb row bleed).
  - fp32r matmuls (full PE rate at N>=256, ~11-bit mantissa rounding).
  - -sigmoid(x) = sigmoid(-x) - 1: the negate rides the ACT scale, the
    -1 and the NaN mask ride one post-transpose DVE add.
"""

import numpy as np

try:  # persistent executable cache: makes repeated cold launches fast
    import jax
    jax.config.update("jax_compilation_cache_dir", "/var/tmp/jax_comp_cache")
    jax.config.update("jax_persistent_cache_min_compile_time_secs", 10)
except Exception:
    pass

import concourse.bacc as bacc
import concourse.mybir as mybir
import concourse.tile as tile
from concourse.alu_op_type import AluOpType
from concourse.bass_utils import run_bass_kernel_spmd

F32 = mybir.dt.float32
F32R = mybir.dt.float32r
AF = mybir.ActivationFunctionType

N_CORES = 8
W = 394                 # image width (stride of the flat layouts)
WV = 384                # valid feature width after 5 VALID 3x3 convs
C = 112                 # conv feature maps
D = 32                  # disparities (-31..0)
ROWS = 24               # output feature rows per core
IN_ROWS = ROWS + 10     # image rows per core
BLOCK_ROWS = 6
NBLK = ROWS // BLOCK_ROWS
BCOLS = BLOCK_ROWS * W  # 2364
PAD = 32                # left pad of L1/R1 so shifted reads stay in-bounds
CHUNK = 512

_cache = {}


def _emit_conv_tower(nc, pool, ps_pool, img_dram, w1T_t, wT_t, spill_dram):
    """Five conv layers for one image slice; result spilled to DRAM."""
    # layer 1 via im2col: rhs [9, 32*W], row t = img[flat + (t//3)*W + t%3]
    rows1 = IN_ROWS - 2
    n1 = rows1 * W
    imc = pool.tile([9, n1], F32R, tag="imc")
    for t in range(9):
        off = (t // 3) * W + (t % 3)
        nc.sync.dma_start(out=imc[t : t + 1, :], in_=img_dram[off : off + n1])
    cur = pool.tile([C, n1 + 8], F32R, tag="feat_a")
    for c0 in range(0, n1, CHUNK):
        n = min(CHUNK, n1 - c0)
        ps = ps_pool.tile([C, 1024], F32, tag="ps")
        nc.tensor.matmul(ps[:, :n], w1T_t[:, :], imc[:, c0 : c0 + n],
                         start=True, stop=True)
        nc.scalar.activation(cur[:, c0 : c0 + n], ps[:C, :n], AF.Relu)

    # layers 2..5
    for li in range(4):
        rows_out = rows1 - 2 * (li + 1)
        n_out = rows_out * W
        tag = ("feat_b", "feat_c", "feat_a", "feat_b")[li]
        out = pool.tile([C, n_out + 8], F32R, tag=tag)
        wT = wT_t[li]
        for c0 in range(0, n_out, CHUNK):
            n = min(CHUNK, n_out - c0)
            ps = ps_pool.tile([C, 1024], F32, tag="ps")
            for t in range(9):
                off = (t // 3) * W + (t % 3)
                nc.tensor.matmul(ps[:, :n], wT[:, t * C : (t + 1) * C],
                                 cur[:, c0 + off : c0 + off + n],
                                 start=(t == 0), stop=(t == 8))
            if li % 2 == 0:
                nc.scalar.activation(out[:, c0 : c0 + n], ps[:C, :n], AF.Relu)
            else:
                nc.vector.tensor_scalar(out=out[:, c0 : c0 + n], in0=ps[:C, :n],
                                        scalar1=0.0, scalar2=None,
                                        op0=AluOpType.max)
        cur = out

    for b in range(NBLK):
        nc.sync.dma_start(out=spill_dram[:, b * BCOLS : (b + 1) * BCOLS],
                          in_=cur[:, b * BCOLS : (b + 1) * BCOLS])


def build_nc():
    nc = bacc.Bacc("TRN2")

    limg = nc.dram_tensor("limg", [IN_ROWS * W + 8], F32R, kind="ExternalInput")
    rimg = nc.dram_tensor("rimg", [IN_ROWS * W + 8], F32R, kind="ExternalInput")
    w1T = nc.dram_tensor("w1T", [9, C], F32R, kind="ExternalInput")
    wT_d = [nc.dram_tensor(f"wT{l}", [C, 9 * C], F32R, kind="ExternalInput")
            for l in range(2, 6)]
    fw1L = nc.dram_tensor("fw1L", [C, 384], F32R, kind="ExternalInput")
    fw1R = nc.dram_tensor("fw1R", [C, 384], F32R, kind="ExternalInput")
    fw2 = nc.dram_tensor("fw2", [128, 3, 384], F32R, kind="ExternalInput")
    fw3 = nc.dram_tensor("fw3", [128, 3, 384], F32R, kind="ExternalInput")
    fw4 = nc.dram_tensor("fw4", [128, 3, 1], F32R, kind="ExternalInput")
    cpost = nc.dram_tensor("cpost", [128, 96], F32, kind="ExternalInput")
    ident = nc.dram_tensor("ident", [32, 32], F32, kind="ExternalInput")
    out_d = nc.dram_tensor("out", [ROWS, WV, D], F32, kind="ExternalOutput")

    lf_spill = nc.dram_tensor("lf_spill", [C, ROWS * W], F32R)
    rf_spill = nc.dram_tensor("rf_spill", [C, ROWS * W], F32R)

    with tile.TileContext(nc) as tc:
        # ---------------- conv phase ----------------
        with tc.tile_pool(name="convw", bufs=1) as wpool, \
             tc.tile_pool(name="conv", bufs=1) as pool, \
             tc.tile_pool(name="convps", bufs=3, space="PSUM") as ps_pool:
            w1T_t = wpool.tile([9, C], F32R)
            nc.sync.dma_start(out=w1T_t[:], in_=w1T[:, :])
            wT_t = []
            for i in range(4):
                t = wpool.tile([C, 9 * C], F32R, tag=f"wT{i}")
                nc.sync.dma_start(out=t[:], in_=wT_d[i][:, :])
                wT_t.append(t)
            _emit_conv_tower(nc, pool, ps_pool, limg, w1T_t, wT_t, lf_spill)
            _emit_conv_tower(nc, pool, ps_pool, rimg, w1T_t, wT_t, rf_spill)

        # ---------------- MLP phase ----------------
        with tc.tile_pool(name="mlpw", bufs=1) as wpool, \
             tc.tile_pool(name="mlp", bufs=1) as pool, \
             tc.tile_pool(name="mlp3", bufs=2) as pool3, \
             tc.tile_pool(name="psbig", bufs=6, space="PSUM") as psb, \
             tc.tile_pool(name="pssmall", bufs=1, space="PSUM") as pss:
            fw1L_t = wpool.tile([C, 384], F32R, tag="fw1L")
            fw1R_t = wpool.tile([C, 384], F32R, tag="fw1R")
            fw2_t = wpool.tile([128, 3, 384], F32R, tag="fw2")
            fw3_t = wpool.tile([128, 3, 384], F32R, tag="fw3")
            fw4_t = wpool.tile([128, 3, 1], F32R, tag="fw4")
            cpost_t = wpool.tile([128, 96], F32, tag="cpost")
            ident_t = wpool.tile([32, 32], F32, tag="ident")
            nc.sync.dma_start(out=fw1L_t[:], in_=fw1L[:, :])
            nc.sync.dma_start(out=fw1R_t[:], in_=fw1R[:, :])
            nc.sync.dma_start(out=fw2_t[:], in_=fw2[:, :, :])
            nc.sync.dma_start(out=fw3_t[:], in_=fw3[:, :, :])
            nc.sync.dma_start(out=fw4_t[:], in_=fw4[:, :, :])
            nc.sync.dma_start(out=cpost_t[:], in_=cpost[:, :])
            nc.sync.dma_start(out=ident_t[:], in_=ident[:, :])

            # fw2_t[k] is the lhsT K-chunk k; M-chunk m is [:, m*128:(m+1)*128]
            fw2_k = [fw2_t[:, k, :] for k in range(3)]
            fw3_k = [fw3_t[:, k, :] for k in range(3)]
            fw4_k = [fw4_t[:, k, :] for k in range(3)]

            for b in range(NBLK):
                c0blk = b * BCOLS
                lf_b = pool.tile([C, BCOLS], F32R, tag="lfb")
                rf_b = pool.tile([C, BCOLS], F32R, tag="rfb")
                nc.sync.dma_start(out=lf_b[:], in_=lf_spill[:, c0blk : c0blk + BCOLS])
                nc.sync.dma_start(out=rf_b[:], in_=rf_spill[:, c0blk : c0blk + BCOLS])

                L1 = pool.tile([128, 3, PAD + BCOLS], F32, tag="L1")
                R1 = pool.tile([128, 3, PAD + BCOLS], F32, tag="R1")
                nc.vector.memset(R1[:, :, 0:PAD], 0.0)

                # stage A: L1 = fw1L.T @ lf_b ; R1 = fw1R.T @ rf_b
                for m in range(3):
                    for dst, wsrc, feat, eng in ((L1, fw1L_t, lf_b, nc.scalar),
                                                 (R1, fw1R_t, rf_b, nc.vector)):
                        for c0 in range(0, BCOLS, CHUNK):
                            n = min(CHUNK, BCOLS - c0)
                            ps = psb.tile([128, 512], F32, tag="ps")
                            nc.tensor.matmul(
                                ps[:, :n],
                                wsrc[:, m * 128 : (m + 1) * 128],
                                feat[:, c0 : c0 + n],
                                start=True, stop=True)
                            if eng is nc.scalar:
                                nc.scalar.activation(
                                    dst[:, m, PAD + c0 : PAD + c0 + n],
                                    ps[:, :n], AF.Identity)
                            else:
                                nc.vector.tensor_copy(
                                    out=dst[:, m, PAD + c0 : PAD + c0 + n],
                                    in_=ps[:, :n])
                score = pool.tile([D, BCOLS], F32, tag="score")

                for di in range(D):
                    doff = di - 31  # the actual disparity shift (<= 0)
                    h1 = pool.tile([128, 3, BCOLS], F32R, tag="h1")
                    HB = BCOLS // 2
                    for k in range(3):
                        for c0 in (0, HB):
                            tmp = pool3.tile([128, HB], F32, tag="tmp")
                            nc.vector.tensor_tensor(
                                out=tmp[:, :],
                                in0=L1[:, k, PAD + c0 : PAD + c0 + HB],
                                in1=R1[:, k, PAD + doff + c0 : PAD + doff + c0 + HB],
                                op=AluOpType.add)
                            nc.vector.tensor_scalar(
                                out=h1[:, k, c0 : c0 + HB], in0=tmp[:, :],
                                scalar1=0.0, scalar2=None, op0=AluOpType.max)

                    # L2: h2 = relu(fw2.T @ h1); L3: h3 = relu(fw3.T @ h2)
                    h2 = pool.tile([128, 3, BCOLS], F32R, tag="h2")
                    h3 = pool.tile([128, 3, BCOLS], F32R, tag="h3")
                    for src, wk, dst, eng in ((h1, fw2_k, h2, "v"),
                                              (h2, fw3_k, h3, "s")):
                        for m in range(3):
                            for c0 in range(0, BCOLS, CHUNK):
                                n = min(CHUNK, BCOLS - c0)
                                ps = psb.tile([128, 512], F32, tag="ps")
                                for k in range(3):
                                    nc.tensor.matmul(
                                        ps[:, :n],
                                        wk[k][:, m * 128 : (m + 1) * 128],
                                        src[:, k, c0 : c0 + n],
                                        start=(k == 0), stop=(k == 2))
                                use_dve = (eng == "v" and m < 1)
                                if use_dve:
                                    nc.vector.tensor_scalar(
                                        out=dst[:, m, c0 : c0 + n],
                                        in0=ps[:, :n],
                                        scalar1=0.0, scalar2=None,
                                        op0=AluOpType.max)
                                else:
                                    nc.scalar.activation(
                                        dst[:, m, c0 : c0 + n],
                                        ps[:, :n], AF.Relu)

                    # L4 + sigmoid(-x): score_di = sigmoid(-(fw4 . h3))
                    # ACT cannot write at partition di, so stage on
                    # partition 0 and DMA the row into place.
                    tmp4 = pool.tile([1, BCOLS], F32, tag="tmp4")
                    for c0 in range(0, BCOLS, 1024):
                        n = min(1024, BCOLS - c0)
                        ps4 = pss.tile([1, 1024], F32, tag="ps4")
                        for cc in range(0, n, CHUNK):
                            nn = min(CHUNK, n - cc)
                            for k in range(3):
                                nc.tensor.matmul(ps4[:, cc : cc + nn], fw4_k[k][:, :],
                                                 h3[:, k, c0 + cc : c0 + cc + nn],
                                                 start=(k == 0), stop=(k == 2))
                        nc.scalar.activation(tmp4[:1, c0 : c0 + n],
                                             ps4[:1, :n], AF.Sigmoid, scale=-1.0)
                    nc.gpsimd.dma_start(out=score[di : di + 1, :], in_=tmp4[:1, :])

                # transpose + mask + store, one output row at a time
                for y in range(BLOCK_ROWS):
                    yg = b * BLOCK_ROWS + y
                    pst = psb.tile([128, 96], F32, tag="ps")
                    for c in range(3):
                        nc.tensor.transpose(
                            pst[:, c * 32 : (c + 1) * 32],
                            score[:, y * W + c * 128 : y * W + (c + 1) * 128],
                            ident_t[:, :])
                    stg = pool3.tile([128, 96], F32, tag="stg")
                    nc.vector.scalar_tensor_tensor(
                        out=stg[:], in0=pst[:], scalar=0.0, in1=cpost_t[:],
                        op0=AluOpType.add, op1=AluOpType.add)
                    nc.sync.dma_start(
                        out=out_d[yg].rearrange("(c p) d -> p c d", p=128),
                        in_=stg[:].rearrange("p (c d) -> p c d", c=3))

    nc.compile()
    return nc


def _prep_shared(inputs):
    f = np.float32
    cw = [np.asarray(inputs[f"cw{l}"], f) for l in range(1, 6)]
    fw = [np.asarray(inputs[f"fw{l}"], f) for l in range(1, 5)]
    for l in range(1, 6):
        assert not np.any(np.asarray(inputs[f"cb{l}"])), "nonzero conv bias"
    for l in range(1, 5):
        assert not np.any(np.asarray(inputs[f"fb{l}"])), "nonzero fc bias"
    assert int(inputs["disp_min"]) == -31 and int(inputs["disp_max"]) == 0

    shared = {
        "w1T": np.ascontiguousarray(cw[0][:, 0].transpose(1, 2, 0).reshape(9, C)),
        "fw1L": np.ascontiguousarray(fw[0][:C]),
        "fw1R": np.ascontiguousarray(fw[0][C:]),
        "fw2": np.ascontiguousarray(fw[1].reshape(3, 128, 384).transpose(1, 0, 2)),
        "fw3": np.ascontiguousarray(fw[2].reshape(3, 128, 384).transpose(1, 0, 2)),
        "fw4": np.ascontiguousarray(fw[3].reshape(3, 128, 1).transpose(1, 0, 2)),
        "ident": np.eye(32, dtype=f),
    }
    for l in range(2, 6):
        shared[f"wT{l}"] = np.ascontiguousarray(
            cw[l - 1].transpose(1, 2, 3, 0).reshape(C, 9 * C))
    cp = np.full((128, 96), np.nan, f)
    for c in range(3):
        for p in range(128):
            x = c * 128 + p
            for i in range(D):
                if x >= 31 - i:
                    cp[p, c * 32 + i] = -1.0
    shared["cpost"] = cp
    return shared


def _run(inputs, trace=False):
    if "nc" not in _cache:
        _cache["nc"] = build_nc()
    nc = _cache["nc"]

    left = np.asarray(inputs["left"], np.float32)
    right = np.asarray(inputs["right"], np.float32)
    shared = _prep_shared(inputs)

    in_maps = []
    for k in range(N_CORES):
        m = dict(shared)
        r0 = k * ROWS
        pad = np.zeros(8, np.float32)
        m["limg"] = np.concatenate([left[r0 : r0 + IN_ROWS].reshape(-1), pad])
        m["rimg"] = np.concatenate([right[r0 : r0 + IN_ROWS].reshape(-1), pad])
        in_maps.append(m)

    r = run_bass_kernel_spmd(nc, in_maps, core_ids=list(range(N_CORES)),
                             trace=trace)
    out = np.concatenate([r.results[k]["out"] for k in range(N_CORES)], axis=0)
    return out, r


def kernel(**inputs) -> np.ndarray:
    out, _ = _run(inputs)
    return out



# revision 26
# speedup vs baseline: 1.4351x; 1.4351x over previous
"""MC-CNN stereo cost-volume inference on 8 Trainium2 NeuronCores.

Sharding: image rows. Core k computes output feature rows [24k, 24k+24)
(input image rows [24k, 24k+34) with conv halo). Each core runs the full
5-layer conv tower on its slice, then the 4-layer MLP for all 32
disparities. Output rows are concatenated on the host.

Key structural tricks:
  - Feature-major layout [C, rows*W] with flat-sliding conv taps; the 2
    garbage columns per row per layer stay right of the valid region.
  - MLP layer 1 is hoisted out of the disparity loop:
      h1_d = relu(fw1L.T@lf + shift(fw1R.T@rf, d))
    L1 = fw1L.T@lf and R1 = fw1R.T@rf are computed once per row-block;
    the shift is a flat offset read of R1 (garbage cols absorb row bleed).
  - fp32r matmuls (full PE rate at N>=256, ~11-bit mantissa rounding).
  - -sigmoid(x) = sigmoid(-x) - 1: the negate rides the ACT scale, the
    -1 and the NaN mask ride one post-transpose DVE add.
"""

import numpy as np

try:  # persistent executable cache: makes repeated cold launches fast
    import jax
    jax.config.update("jax_compilation_cache_dir", "/var/tmp/jax_comp_cache")
    jax.config.update("jax_persistent_cache_min_compile_time_secs", 10)
except Exception:
    pass

import concourse.bacc as bacc
import concourse.mybir as mybir
import concourse.tile as tile
from concourse.alu_op_type import AluOpType
from concourse.bass_utils import run_bass_kernel_spmd

F32 = mybir.dt.float32
F32R = mybir.dt.float32r
AF = mybir.ActivationFunctionType

N_CORES = 8
W = 394                 # image width (stride of the flat layouts)
WV = 384                # valid feature width after 5 VALID 3x3 convs
C = 112                 # conv feature maps
D = 32                  # disparities (-31..0)
ROWS = 24               # output feature rows per core
IN_ROWS = ROWS + 10     # image rows per core
BLOCK_ROWS = 6
NBLK = ROWS // BLOCK_ROWS
BCOLS = BLOCK_ROWS * W  # 2364
PAD = 32                # left pad of L1/R1 so shifted reads stay in-bounds
SB = BLOCK_ROWS * WV    # 2304: compact (garbage-free) block width
CHUNK = 512

_cache = {}


def _emit_conv_tower(nc, pool, ps_pool, img_dram, w1T_t, wT_t, spill_dram):
    """Five conv layers for one image slice; result spilled to DRAM."""
    # layer 1 via im2col: rhs [9, 32*W], row t = img[flat + (t//3)*W + t%3]
    rows1 = IN_ROWS - 2
    n1 = rows1 * W
    imc = pool.tile([9, n1], F32R, tag="imc")
    for t in range(9):
        off = (t // 3) * W + (t % 3)
        nc.sync.dma_start(out=imc[t : t + 1, :], in_=img_dram[off : off + n1])
    cur = pool.tile([C, n1 + 8], F32R, tag="feat_a")
    for c0 in range(0, n1, CHUNK):
        n = min(CHUNK, n1 - c0)
        ps = ps_pool.tile([C, 1024], F32, tag="ps")
        nc.tensor.matmul(ps[:, :n], w1T_t[:, :], imc[:, c0 : c0 + n],
                         start=True, stop=True)
        nc.scalar.activation(cur[:, c0 : c0 + n], ps[:C, :n], AF.Relu)

    # layers 2..5
    for li in range(4):
        rows_out = rows1 - 2 * (li + 1)
        n_out = rows_out * W
        tag = ("feat_b", "feat_c", "feat_a", "feat_b")[li]
        out = pool.tile([C, n_out + 8], F32R, tag=tag)
        wT = wT_t[li]
        for c0 in range(0, n_out, CHUNK):
            n = min(CHUNK, n_out - c0)
            ps = ps_pool.tile([C, 1024], F32, tag="ps")
            for t in range(9):
                off = (t // 3) * W + (t % 3)
                nc.tensor.matmul(ps[:, :n], wT[:, t * C : (t + 1) * C],
                                 cur[:, c0 + off : c0 + off + n],
                                 start=(t == 0), stop=(t == 8))
            if li % 2 == 0:
                nc.scalar.activation(out[:, c0 : c0 + n], ps[:C, :n], AF.Relu)
            else:
                nc.vector.tensor_scalar(out=out[:, c0 : c0 + n], in0=ps[:C, :n],
                                        scalar1=0.0, scalar2=None,
                                        op0=AluOpType.max)
        cur = out

    for b in range(NBLK):
        nc.sync.dma_start(out=spill_dram[:, b * BCOLS : (b + 1) * BCOLS],
                          in_=cur[:, b * BCOLS : (b + 1) * BCOLS])


def build_nc():
    nc = bacc.Bacc("TRN2")

    limg = nc.dram_tensor("limg", [IN_ROWS * W + 8], F32R, kind="ExternalInput")
    rimg = nc.dram_tensor("rimg", [IN_ROWS * W + 8], F32R, kind="ExternalInput")
    w1T = nc.dram_tensor("w1T", [9, C], F32R, kind="ExternalInput")
    wT_d = [nc.dram_tensor(f"wT{l}", [C, 9 * C], F32R, kind="ExternalInput")
            for l in range(2, 6)]
    fw1L = nc.dram_tensor("fw1L", [C, 384], F32R, kind="ExternalInput")
    fw1R = nc.dram_tensor("fw1R", [C, 384], F32R, kind="ExternalInput")
    fw2 = nc.dram_tensor("fw2", [128, 3, 384], F32R, kind="ExternalInput")
    fw3 = nc.dram_tensor("fw3", [128, 3, 384], F32R, kind="ExternalInput")
    fw4 = nc.dram_tensor("fw4", [128, 3, 1], F32R, kind="ExternalInput")
    cpost = nc.dram_tensor("cpost", [128, 96], F32, kind="ExternalInput")
    ident = nc.dram_tensor("ident", [32, 32], F32, kind="ExternalInput")
    out_d = nc.dram_tensor("out", [ROWS, WV, D], F32, kind="ExternalOutput")

    lf_spill = nc.dram_tensor("lf_spill", [C, ROWS * W], F32R)
    rf_spill = nc.dram_tensor("rf_spill", [C, ROWS * W], F32R)

    with tile.TileContext(nc) as tc:
        # ---------------- conv phase ----------------
        with tc.tile_pool(name="convw", bufs=1) as wpool, \
             tc.tile_pool(name="conv", bufs=1) as pool, \
             tc.tile_pool(name="convps", bufs=3, space="PSUM") as ps_pool:
            w1T_t = wpool.tile([9, C], F32R)
            nc.sync.dma_start(out=w1T_t[:], in_=w1T[:, :])
            wT_t = []
            for i in range(4):
                t = wpool.tile([C, 9 * C], F32R, tag=f"wT{i}")
                nc.sync.dma_start(out=t[:], in_=wT_d[i][:, :])
                wT_t.append(t)
            _emit_conv_tower(nc, pool, ps_pool, limg, w1T_t, wT_t, lf_spill)
            _emit_conv_tower(nc, pool, ps_pool, rimg, w1T_t, wT_t, rf_spill)

        # ---------------- MLP phase ----------------
        with tc.tile_pool(name="mlpw", bufs=1) as wpool, \
             tc.tile_pool(name="mlp", bufs=1) as pool, \
             tc.tile_pool(name="mlp3", bufs=2) as pool3, \
             tc.tile_pool(name="psbig", bufs=3, space="PSUM") as psb, \
             tc.tile_pool(name="pssmall", bufs=1, space="PSUM") as pss:
            fw1L_t = wpool.tile([C, 384], F32R, tag="fw1L")
            fw1R_t = wpool.tile([C, 384], F32R, tag="fw1R")
            fw2_t = wpool.tile([128, 3, 384], F32R, tag="fw2")
            fw3_t = wpool.tile([128, 3, 384], F32R, tag="fw3")
            fw4_t = wpool.tile([128, 3, 1], F32R, tag="fw4")
            cpost_t = wpool.tile([128, 96], F32, tag="cpost")
            ident_t = wpool.tile([32, 32], F32, tag="ident")
            nc.sync.dma_start(out=fw1L_t[:], in_=fw1L[:, :])
            nc.sync.dma_start(out=fw1R_t[:], in_=fw1R[:, :])
            nc.sync.dma_start(out=fw2_t[:], in_=fw2[:, :, :])
            nc.sync.dma_start(out=fw3_t[:], in_=fw3[:, :, :])
            nc.sync.dma_start(out=fw4_t[:], in_=fw4[:, :, :])
            nc.sync.dma_start(out=cpost_t[:], in_=cpost[:, :])
            nc.sync.dma_start(out=ident_t[:], in_=ident[:, :])

            # fw2_t[k] is the lhsT K-chunk k; M-chunk m is [:, m*128:(m+1)*128]
            fw2_k = [fw2_t[:, k, :] for k in range(3)]
            fw3_k = [fw3_t[:, k, :] for k in range(3)]
            fw4_k = [fw4_t[:, k, :] for k in range(3)]

            for b in range(NBLK):
                c0blk = b * BCOLS
                lf_b = pool.tile([C, BCOLS], F32R, tag="lfb")
                rf_b = pool.tile([C, BCOLS], F32R, tag="rfb")
                nc.sync.dma_start(out=lf_b[:], in_=lf_spill[:, c0blk : c0blk + BCOLS])
                nc.sync.dma_start(out=rf_b[:], in_=rf_spill[:, c0blk : c0blk + BCOLS])

                L1 = pool.tile([128, 3, PAD + BCOLS], F32, tag="L1")
                R1 = pool.tile([128, 3, PAD + BCOLS], F32, tag="R1")
                nc.vector.memset(R1[:, :, 0:PAD], 0.0)

                # stage A: L1 = fw1L.T @ lf_b ; R1 = fw1R.T @ rf_b
                for m in range(3):
                    for dst, wsrc, feat, eng in ((L1, fw1L_t, lf_b, nc.scalar),
                                                 (R1, fw1R_t, rf_b, nc.vector)):
                        for c0 in range(0, BCOLS, CHUNK):
                            n = min(CHUNK, BCOLS - c0)
                            ps = psb.tile([128, 512], F32, tag="ps")
                            nc.tensor.matmul(
                                ps[:, :n],
                                wsrc[:, m * 128 : (m + 1) * 128],
                                feat[:, c0 : c0 + n],
                                start=True, stop=True)
                            if eng is nc.scalar:
                                nc.scalar.activation(
                                    dst[:, m, PAD + c0 : PAD + c0 + n],
                                    ps[:, :n], AF.Identity)
                            else:
                                nc.vector.tensor_copy(
                                    out=dst[:, m, PAD + c0 : PAD + c0 + n],
                                    in_=ps[:, :n])
                score = pool.tile([D, SB], F32, tag="score")
                h3z = pool.tile([128, 3, SB], F32R, tag="h3")
                nc.vector.memset(h3z[:, :, :].bitcast(F32), 0.0)

                for di in range(D):
                    doff = di - 31  # the actual disparity shift (<= 0)
                    h1 = pool.tile([128, 3, BCOLS], F32R, tag="h1")
                    HB = BCOLS // 2
                    for k in range(3):
                        for c0 in (0, HB):
                            tmp = pool3.tile([128, HB], F32, tag="tmp")
                            nc.vector.tensor_tensor(
                                out=tmp[:, :],
                                in0=L1[:, k, PAD + c0 : PAD + c0 + HB],
                                in1=R1[:, k, PAD + doff + c0 : PAD + doff + c0 + HB],
                                op=AluOpType.add)
                            nc.vector.tensor_scalar(
                                out=h1[:, k, c0 : c0 + HB], in0=tmp[:, :],
                                scalar1=0.0, scalar2=None, op0=AluOpType.max)

                    # L2/L3 skip the per-disparity invalid left margin
                    # (cols x < 31-di are NaN-masked in the output) and the
                    # 10 garbage cols per row: matmul windows are
                    # [row*stride + m0, row*stride + 384), evac'd pairwise
                    # into compact 384-stride h2/h3.
                    m0 = (31 - di) & ~1  # even: fp32r matmuls need 8B-aligned APs
                    nwin = WV - m0
                    h2 = pool.tile([128, 3, SB], F32R, tag="h2")
                    h3 = pool.tile([128, 3, SB], F32R, tag="h3")
                    for src, wk, dst, eng, stride in (
                            (h1, fw2_k, h2, "v", W),
                            (h2, fw3_k, h3, "s", WV)):
                        for m in range(3):
                            for r0 in range(0, BLOCK_ROWS, 2):
                                ps = psb.tile([128, 1024], F32, tag="ps")
                                for rr in range(2):
                                    r = r0 + rr
                                    w0 = r * stride + m0
                                    for k in range(3):
                                        # rr windows at 512-aligned bases:
                                        # a matmul output must not cross a
                                        # PSUM bank boundary
                                        nc.tensor.matmul(
                                            ps[:, rr * 512 + m0 : rr * 512 + WV],
                                            wk[k][:, m * 128 : (m + 1) * 128],
                                            src[:, k, w0 : w0 + nwin],
                                            start=(k == 0), stop=(k == 2))
                                psv = ps[:, :].rearrange(
                                    "p (r x) -> p r x", x=512)[:, :, m0:WV]
                                dstv = dst[:, m, r0 * WV : (r0 + 2) * WV].rearrange(
                                    "p (r x) -> p r x", x=WV)[:, :, m0:]
                                use_dve = (eng == "v" and m < 1)
                                if use_dve:
                                    nc.vector.tensor_scalar(
                                        out=dstv, in0=psv,
                                        scalar1=0.0, scalar2=None,
                                        op0=AluOpType.max)
                                else:
                                    nc.scalar.activation(dstv, psv, AF.Relu)

                    # L4 + sigmoid(-x) over full compact rows (stale margins
                    # feed only NaN-masked outputs)
                    tmp4 = pool.tile([1, SB], F32, tag="tmp4")
                    for c0 in range(0, SB, 1024):
                        n = min(1024, SB - c0)
                        ps4 = pss.tile([1, 1024], F32, tag="ps4")
                        for cc in range(0, n, CHUNK):
                            nn = min(CHUNK, n - cc)
                            for k in range(3):
                                nc.tensor.matmul(ps4[:, cc : cc + nn], fw4_k[k][:, :],
                                                 h3[:, k, c0 + cc : c0 + cc + nn],
                                                 start=(k == 0), stop=(k == 2))
                        nc.scalar.activation(tmp4[:1, c0 : c0 + n],
                                             ps4[:1, :n], AF.Sigmoid, scale=-1.0)
                    nc.gpsimd.dma_start(out=score[di : di + 1, :], in_=tmp4[:1, :])

                # transpose + mask + store, one output row at a time
                for y in range(BLOCK_ROWS):
                    yg = b * BLOCK_ROWS + y
                    pst = psb.tile([128, 96], F32, tag="ps")
                    for c in range(3):
                        nc.tensor.transpose(
                            pst[:, c * 32 : (c + 1) * 32],
                            score[:, y * WV + c * 128 : y * WV + (c + 1) * 128],
                            ident_t[:, :])
                    stg = pool3.tile([128, 96], F32, tag="stg")
                    nc.vector.scalar_tensor_tensor(
                        out=stg[:], in0=pst[:], scalar=0.0, in1=cpost_t[:],
                        op0=AluOpType.add, op1=AluOpType.add)
                    nc.sync.dma_start(
                        out=out_d[yg].rearrange("(c p) d -> p c d", p=128),
                        in_=stg[:].rearrange("p (c d) -> p c d", c=3))

    nc.compile()
    return nc


def _prep_shared(inputs):
    f = np.float32
    cw = [np.asarray(inputs[f"cw{l}"], f) for l in range(1, 6)]
    fw = [np.asarray(inputs[f"fw{l}"], f) for l in range(1, 5)]
    for l in range(1, 6):
        assert not np.any(np.asarray(inputs[f"cb{l}"])), "nonzero conv bias"
    for l in range(1, 5):
        assert not np.any(np.asarray(inputs[f"fb{l}"])), "nonzero fc bias"
    assert int(inputs["disp_min"]) == -31 and int(inputs["disp_max"]) == 0

    shared = {
        "w1T": np.ascontiguousarray(cw[0][:, 0].transpose(1, 2, 0).reshape(9, C)),
        "fw1L": np.ascontiguousarray(fw[0][:C]),
        "fw1R": np.ascontiguousarray(fw[0][C:]),
        "fw2": np.ascontiguousarray(fw[1].reshape(3, 128, 384).transpose(1, 0, 2)),
        "fw3": np.ascontiguousarray(fw[2].reshape(3, 128, 384).transpose(1, 0, 2)),
        "fw4": np.ascontiguousarray(fw[3].reshape(3, 128, 1).transpose(1, 0, 2)),
        "ident": np.eye(32, dtype=f),
    }
    for l in range(2, 6):
        shared[f"wT{l}"] = np.ascontiguousarray(
            cw[l - 1].transpose(1, 2, 3, 0).reshape(C, 9 * C))
    cp = np.full((128, 96), np.nan, f)
    for c in range(3):
        for p in range(128):
            x = c * 128 + p
            for i in range(D):
                if x >= 31 - i:
                    cp[p, c * 32 + i] = -1.0
    shared["cpost"] = cp
    return shared


def _run(inputs, trace=False):
    if "nc" not in _cache:
        _cache["nc"] = build_nc()
    nc = _cache["nc"]

    left = np.asarray(inputs["left"], np.float32)
    right = np.asarray(inputs["right"], np.float32)
    shared = _prep_shared(inputs)

    in_maps = []
    for k in range(N_CORES):
        m = dict(shared)
        r0 = k * ROWS
        pad = np.zeros(8, np.float32)
        m["limg"] = np.concatenate([left[r0 : r0 + IN_ROWS].reshape(-1), pad])
        m["rimg"] = np.concatenate([right[r0 : r0 + IN_ROWS].reshape(-1), pad])
        in_maps.append(m)

    r = run_bass_kernel_spmd(nc, in_maps, core_ids=list(range(N_CORES)),
                             trace=trace)
    out = np.concatenate([r.results[k]["out"] for k in range(N_CORES)], axis=0)
    return out, r


def kernel(**inputs) -> np.ndarray:
    out, _ = _run(inputs)
    return out

